# revision 16
# baseline (speedup 1.0000x reference)
"""Trainium2 Bass kernel for nn_LCRLoss (weighted BCE + co-occurrence loss).

Math notes (derivation):
  out = mean(ce) + LAMBDA * co_loss / C^2
  ce = -(pos_w * t * log(sigmoid(x)+eps) + neg_w * (1-t) * log(1-sigmoid(x)+eps))
  With |x| <~ 6 the eps shifts are < 3e-6 absolute, so
     log(sigmoid(x))   = x - softplus(x)
     log(1-sigmoid(x)) = -softplus(x)
  ce element = -pos_w*t*(x - sp) + neg_w*(1-t)*sp       where sp = softplus(x)
  sum(ce)    = sum_c [ -pos_w[c]*(cs_tx - cs_tsp) + neg_w[c]*(cs_sp - cs_tsp) ]
     with column sums cs_q = sum_b q[b,c] for q in {t*x, t*sp, sp}.

  co_loss = sum_{i!=j} C[i,j]*(s2[i]+s2[j]-2*gram[i,j])
          = dot(s2, R+S) - 2*<C, gram>          (diagonal cancels exactly)
  with gram = X^T X / B, s2 = diag(X^T X)/B, R/S = row/col sums of C:
  co_loss = < M, X^T X > / B   where  M = diag(R+S) - 2*C   (host precomputed)

Sharding: data-parallel over B across 8 cores (1024 rows each). Every
per-core output is a partial sum; host combines in float64.

Device per core:
  - x: f32 HBM -> bf16 SBUF (SWDGE cast DMA), t: int32 -> bf16 on DVE
  - ACT: sp = Softplus(x)
  - DVE: tx = t*x, tsp = t*sp (bf16)
  - PE: column sums of tx/tsp/sp via ones-vector matmuls (PSUM accumulate)
  - PE: G = X^T X per 128x512 block (bf16, f32 PSUM accumulate over b)
  - DVE: tensor_tensor_reduce (M .* G -> free-axis sums) -> [128, 16]
  - outputs: out_cs [1, 3072] f32, out_wg [128, 16] f32
"""

import os
import sys

for _p in ("/opt/trn_rl_repo",):
    if _p not in sys.path and os.path.isdir(_p):
        sys.path.insert(0, _p)

import numpy as np
import ml_dtypes

import bass_rust
import concourse.bass as bass
import concourse.mybir as mybir
import concourse.tile as tile
from concourse.vector_clock import ScopedClock


def _split_drain_and_barrier(self, tick_clock, wait_clock):
    """Replacement for TileContext._drain_and_barrier.

    The stock version stacks every outstanding semaphore wait onto the single
    kernel-tail Drain; this walrus build caps sync-wait commands per
    instruction, so spread the waits across a chain of 1-wait carrier drains
    emitted before the sem-clearing barrier."""
    drain_inst = self.nc.sync.drain()
    wait_clock.add_sem_waits(
        drain_inst.ins, ScopedClock({None: tick_clock.global_clock})
    )
    ins = drain_inst.ins
    si = ins.sync_info
    if si is not None and si.on_wait and len(si.on_wait) > 1:
        waits = list(si.on_wait)
        ins.sync_info = bass_rust.SyncInfo(
            on_wait=[waits[0]], on_update=list(si.on_update)
        )
        for w in waits[1:]:
            carrier = self.nc.sync.drain()
            carrier.ins.sync_info = bass_rust.SyncInfo(on_wait=[w], on_update=[])

    self.nc.all_engine_barrier()
    assert self.sems is not None
    popped = self.nc._tile_sem_poison_stack.pop()
    assert popped is self._sem_poison
    self.nc.clear_and_free_semaphores(list(self.sems.allocated().values()))
    self.nc.all_engine_barrier()


tile.TileContext._drain_and_barrier = _split_drain_and_barrier

B, C = 8192, 1024
NCORES = 8
BSH = B // NCORES          # rows per core
NB = BSH // 128            # b-tiles per core
MI = C // 128              # gram row-chunks
NJ = C // 512              # gram col-chunks (psum-bank sized)
LAMBDA_CO = 0.02

F32 = mybir.dt.float32
BF16 = mybir.dt.bfloat16
I32 = mybir.dt.int32

# softplus via native ACT Softplus (1 pass, not in CoreSim) vs Exp+Ln (2 passes)
USE_SOFTPLUS = os.environ.get("LCR_SOFTPLUS", "0") == "1"


def build_nc():
    nc = bass.Bass()
    x = nc.dram_tensor("x", [BSH, C], F32, kind="ExternalInput")
    t = nc.dram_tensor("t", [BSH, C], I32, kind="ExternalInput")
    m = nc.dram_tensor("m", [C, C], BF16, kind="ExternalInput")
    out_cs = nc.dram_tensor("out_cs", [1, 3 * C], F32, kind="ExternalOutput")
    out_wg = nc.dram_tensor("out_wg", [128, MI * NJ], F32, kind="ExternalOutput")

    with tile.TileContext(nc) as tc:
        with (
            tc.tile_pool(name="big", bufs=1) as big,
            tc.tile_pool(name="work", bufs=3) as work,
            tc.tile_pool(name="pscs", bufs=1, space="PSUM") as pscs,
            tc.tile_pool(name="psg", bufs=2, space="PSUM") as psg,
        ):
            ones = big.tile([128, 1], BF16)
            nc.vector.memset(ones, 1.0)

            xbf = big.tile([128, NB, C], BF16)
            mt = big.tile([128, MI, C], BF16)
            sp = big.tile([128, NB, C], BF16)
            tbf = big.tile([128, NB, C], BF16)
            tx = big.tile([128, NB, C], BF16)
            tsp = big.tile([128, NB, C], BF16)
            # TTR main outputs (never read back; distinct slices avoid WAR
            # waits — this walrus build allows only 1 sync-wait per DVE inst)
            scr = big.tile([128, MI * NJ, 512], BF16)
            mobs = big.tile([128, 2], BF16)
            xobs = big.tile([128, NB // 2], BF16)
            wg = big.tile([128, MI * NJ], F32)
            stage = big.tile([1, 3 * C], F32)

            ti = big.tile([128, NB, C], I32)

            # one DMA per tensor: each still fans across all 16 SDMA engine
            # slots (full BW), and the kernel-tail drain's sync-wait list
            # stays under the per-instruction wait-slot cap.
            CH = NB
            x_r = x.rearrange("(nb p) c -> p nb c", p=128)
            t_r = t.rearrange("(nb p) c -> p nb c", p=128)
            m_r = m.rearrange("(mi p) c -> p mi c", p=128)
            # logits: cast f32 -> bf16 during DMA (SWDGE)
            nc.gpsimd.dma_start(out=xbf[:, :, :], in_=x_r)
            # targets: raw int32, converted on DVE below
            nc.sync.dma_start(out=ti[:, :, :], in_=t_r)
            # M (co-matrix combined weights), bf16 direct
            nc.sync.dma_start(out=mt[:, :, :], in_=m_r)

            # PSUM column-sum accumulators: 3 quantities x 2 halves
            cs = {}
            for qi in range(3):
                for h in range(NJ):
                    cs[(qi, h)] = pscs.tile([1, 512], F32, tag=f"cs{qi}{h}", name=f"cs{qi}{h}")

            # Phase A: elementwise + column-sum matmuls
            for b in range(NB):
                if b % CH == 0:
                    # absorb the xbf chunk-DMA wait into a 1-wait DVE no-op so
                    # the muls below never need 2 sync waits
                    c = b // CH
                    nc.vector.tensor_copy(out=xobs[:, c:c + 1], in_=xbf[:, b, 0:1])
                if USE_SOFTPLUS:
                    nc.scalar.activation(
                        out=sp[:, b, :], in_=xbf[:, b, :],
                        func=mybir.ActivationFunctionType.Softplus,
                    )
                else:
                    # sp = ln(exp(x) + 1); Exp and Ln share one ACT table set
                    e = work.tile([128, C], F32, tag="e")
                    nc.scalar.activation(
                        out=e, in_=xbf[:, b, :],
                        func=mybir.ActivationFunctionType.Exp,
                    )
                    nc.scalar.activation(
                        out=sp[:, b, :], in_=e,
                        func=mybir.ActivationFunctionType.Ln, bias=1.0,
                    )
                nc.vector.tensor_copy(out=tbf[:, b, :], in_=ti[:, b, :])
                nc.vector.tensor_mul(out=tx[:, b, :], in0=tbf[:, b, :], in1=xbf[:, b, :])
                nc.vector.tensor_mul(out=tsp[:, b, :], in0=tbf[:, b, :], in1=sp[:, b, :])
                for qi, qt in enumerate((tx, tsp, sp)):
                    for h in range(NJ):
                        nc.tensor.matmul(
                            cs[(qi, h)], ones, qt[:, b, h * 512:(h + 1) * 512],
                            start=(b == 0), stop=(b == NB - 1),
                        )

            # make DVE observe the M DMA before the first phase-B mul, so
            # those muls only ever wait on the PE semaphore (1-wait limit)
            nc.vector.tensor_copy(out=mobs[:, 0:1], in_=mt[:, 0, 0:1])

            # Phase B: gram blocks + <M, G> partial reduction
            for mi in range(MI):
                for nj in range(NJ):
                    g = psg.tile([128, 512], F32)
                    for b in range(NB):
                        nc.tensor.matmul(
                            g,
                            xbf[:, b, mi * 128:(mi + 1) * 128],
                            xbf[:, b, nj * 512:(nj + 1) * 512],
                            start=(b == 0), stop=(b == NB - 1),
                        )
                    k = mi * NJ + nj
                    nc.vector.tensor_mul(
                        out=scr[:, k, :], in0=g,
                        in1=mt[:, mi, nj * 512:(nj + 1) * 512],
                    )
                    nc.vector.reduce_sum(
                        out=wg[:, k:k + 1], in_=scr[:, k, :],
                        axis=mybir.AxisListType.X,
                    )

            # evacuate colsum PSUM -> SBUF staging -> DRAM
            for qi in range(3):
                for h in range(NJ):
                    o = qi * C + h * 512
                    nc.scalar.copy(out=stage[:, o:o + 512], in_=cs[(qi, h)])
            # outputs via SWDGE so completion lands on already-used DMA lanes
            nc.gpsimd.dma_start(out=out_cs[:, :], in_=stage)
            nc.gpsimd.dma_start(out=out_wg[:, :], in_=wg)
    return nc


_NC_CACHE = None


def _get_nc():
    global _NC_CACHE
    if _NC_CACHE is None:
        _NC_CACHE = build_nc()
    return _NC_CACHE


def host_combine(results, label_priors):
    """Combine per-core partial sums (float64) into the final scalar."""
    pri = np.asarray(label_priors).astype(np.float64)
    cs_tx = np.zeros(C, dtype=np.float64)
    cs_tsp = np.zeros(C, dtype=np.float64)
    cs_sp = np.zeros(C, dtype=np.float64)
    wg_sum = 0.0
    for r in results:
        oc = np.asarray(r["out_cs"]).astype(np.float64).reshape(3 * C)
        cs_tx += oc[0:C]
        cs_tsp += oc[C:2 * C]
        cs_sp += oc[2 * C:3 * C]
        wg_sum += np.asarray(r["out_wg"]).astype(np.float64).sum()
    pos_w = 1.0 / pri
    neg_w = 1.0 / (1.0 - pri)
    ce_sum = (-pos_w * (cs_tx - cs_tsp) + neg_w * (cs_sp - cs_tsp)).sum()
    co_loss = wg_sum / B
    total = ce_sum / (B * C) + LAMBDA_CO * co_loss / (C * C)
    return np.array(total, dtype=np.float32)


def make_m_bf16(co_matrix):
    co64 = np.asarray(co_matrix).astype(np.float64)
    R = co64.sum(axis=1)
    S = co64.sum(axis=0)
    M = -2.0 * co64
    M[np.arange(C), np.arange(C)] += R + S
    return M.astype(ml_dtypes.bfloat16)


def make_in_maps(logits, targets, co_matrix):
    logits = np.ascontiguousarray(np.asarray(logits), dtype=np.float32)
    targets = np.ascontiguousarray(np.asarray(targets), dtype=np.int32)
    m_bf = make_m_bf16(co_matrix)
    in_maps = []
    for k in range(NCORES):
        in_maps.append({
            "x": np.ascontiguousarray(logits[k * BSH:(k + 1) * BSH]),
            "t": np.ascontiguousarray(targets[k * BSH:(k + 1) * BSH]),
            "m": m_bf,
        })
    return in_maps


def kernel(logits, targets, co_matrix, label_priors):
    from concourse.bass_utils import run_bass_kernel_spmd

    nc = _get_nc()
    in_maps = make_in_maps(logits, targets, co_matrix)
    res = run_bass_kernel_spmd(nc, in_maps, core_ids=list(range(NCORES)))
    if res.exec_time_ns is not None:
        print(f"HW exec time: {res.exec_time_ns} ns")
    return host_combine(res.results, label_priors)
